# revision 46
# baseline (speedup 1.0000x reference)
"""CDGM (graph-construction GNN) fused kernel for Trainium2, 8-way row-sharded.

Math per layer (reference):
    gl   = relu(x @ Wgl + bgl)                      [N, F]
    t_ij = ||gl_i - gl_j||^2  (via sq_i + sq_j - 2 gl_i.gl_j)
    adj  = sigmoid(-(1+temp)*sqrt(relu(t)+eps) * (t>0) + (5+theta))
    x    = (adj @ (x @ Wgnn + bgnn)) / rowsum(adj)   (+relu except last layer)
    out  = softmax(x)

Device strategy (per core r, query rows Q_r = [1024*r, 1024*(r+1))):
  - glTs = sqrt(2)*gl stored transposed [F, N] fp16; t' = -t accumulated in
    PSUM f32 as (glTs_j . glTs_q) - sq_j - sq_q (K=F fp16 matmul + K=2
    rank-2 correction for layer 0; layer 1 folds both rank-1 terms into the
    main matmul, K=66). sq comes from the same fp16-quantized glTs so the
    cancellation is consistent.
  - ACT reads the PSUM t' tiles directly: sqrt(-1*t' + DELTA) -> fp16 slab.
    The free affine (scale=-1, bias=DELTA) replaces the old DVE clamp pass;
    DELTA=0.02 keeps the sqrt domain non-negative (max positive t'
    excursion measured 0.0073) at ~1e-3 adjacency error.
  - ACT phases are strictly batched per layer (all sqrt, then all sigmoid)
    so the table set switches exactly 4 times total.
  - adj@h runs transposed on both layers: yT[f, q] += h_jc^T @ slab_jc
    (h chunks are the stationary operand, two M=65 tiles for layer 0's
    129 outputs; deg rides as row 64 via a ones column in h). The
    division uses reciprocal_approx_fast + a PE ones-broadcast +
    scalar_tensor_tensor, writing x1T directly in the transposed layout
    the next layer wants - no PE transposes.
  - The post-sigmoid adjacency is written as fp8e4m3 (in place, via a
    bitcast view into the low half of the fp16 slab - the write byte
    index trails the read byte index so the cast is race-free), and h is
    stored fp8 in 80-wide chunks; adj@h then runs as fp8 DoubleRow
    matmuls with K=256 (two j-chunks per instruction). Quantization
    noise averages out over the 8192-term reductions (~2.8e-4 final
    rel err vs 2.1e-4 all-fp16).
  - Matmul shape discipline (measured on hardware): K and the rounded
    M tile size must be uniform across consecutive matmuls feeding open
    PSUM accumulation groups, and K must be exactly 128 - K=66 runs at
    427ns vs 216ns for the same N=512. Hence the rank-2 correction
    tensors are zero-padded to full 128 rows and both adj@h stationary
    tiles are M=65.
  - One AllGather of x1T (256 KB/core fp16) bridges the layers (two
    collectives pay the ~30us CC-core startup serially and can stall
    against concurrent compute). Layer 1 consumes gathered columns
    evens-first; its h is built after the t' loop so it never delays
    the first sqrt.
  - SBUF liveness: the two 32K-wide slabs (128 KB/partition) are global;
    layer-0 streams xT through triple-buffered 1024-col blocks and
    scopes glTs/aug tiles in a pool that closes after the t' loop, so
    layer-1's setup allocates into space whose last readers finished by
    mid-layer-0 (keeps the layer boundary pipelined). h_a8/h_b8 are
    global because their last reader is layer-0's final y matmul.
  - Final division + softmax run on host (y and deg returned raw in f32).

The harness calls kernel(**inputs) with full inputs; sharding is internal.
"""

import math

import numpy as np

N = 8192
D_IN = 256
F0, F1 = 128, 64
N_CORES = 8
QR = N // N_CORES          # query rows per core
NJ = N // 128              # 64 j-chunks of 128
SLABW = NJ * 512           # slab free width per q-tile (32768)
DELTA = 0.02               # sqrt domain guard (see module docstring)

# DVE NR-sqrt offload: PSUM now holds u = t + DELTA >= 0 directly (the
# q-side gl is negated via negated weights), so some sqrt tiles can run on
# the otherwise-idle Vector engine instead of ACT: a stock int32
# tensor_scalar makes the classic bit-trick rsqrt seed (K - bits/2), and a
# 6-stage fused custom op runs one generalized Newton step and multiplies
# by u: s = u*y0*(C0 - C1*u*y0^2). Max rel err 1.26e-3 on sqrt; end-to-end
# effect validated at <1e-5 on the final softmax (the y/deg ratio cancels
# the one-sided NR bias).
K_SEED = float(0x5F65209F)
C0_NR = 1.1571034448217297
C1_NR = 0.2294640697215849
# per-round offload counts (of 32 tiles per round): round-1 DVE work can
# hide under round-0's sigmoid, so it gets a bigger share
OFF_L0 = (0, 0)
OFF_L1 = (8, 12)

_CACHE = {}
_NR_OP = None


def _offload_set(counts):
    out = set()
    for rnd, n in enumerate(counts):
        out |= {rnd * 32 + i for i in range(32)
                if (i + 1) * n // 32 > i * n // 32}
    return out


def _register_nr_sqrt():
    global _NR_OP
    if _NR_OP is not None:
        return _NR_OP
    import concourse.dve_ops as dve_ops
    from concourse.dve_spec import Spec, Src0, Src1, C0, C1, lower, sq, _has_src1
    from concourse.dve_uop import DveOpSpec

    for op in dve_ops.OPS:
        if op.name == "NR_SQRT_ANT":
            _NR_OP = op
            return op
    body = (Src0 * (C0 - C1 * (Src1 * sq(Src0)))) * Src1
    spec = Spec(
        body=body,
        reference=lambda in0, in1, s0, s1, imm2:
            (in0 * (s0 - s1 * (in1 * in0 * in0))) * in1,
    )
    row = dve_ops._CUSTOM_DVE_ROW_BASE + len(dve_ops.OPS)
    shas = {}
    for ver in ("v3", "v4"):
        shas[ver] = DveOpSpec(
            name="NR_SQRT_ANT", opcode=row, uops=lower(spec, ver=ver),
            rd1_en=_has_src1(spec),
        ).sha(ver)
    op = dve_ops.DveOp("NR_SQRT_ANT", spec, subdim=False, uops_sha=shas)
    dve_ops.OPS.append(op)
    dve_ops._SUB_OPCODE_FOR_NAME[op.name] = row
    dve_ops.CUSTOM_DVE_SPECS[op.name] = op.spec
    _NR_OP = op
    return op


def _build(temp: float, theta: float):
    nrop = _register_nr_sqrt()
    import concourse.bacc as bacc
    import concourse.mybir as mybir
    import concourse.tile as tile
    from concourse.tile_rust import add_dep_helper
    from contextlib import ExitStack

    DT = mybir.dt
    AF = mybir.ActivationFunctionType
    ALU = mybir.AluOpType
    F32, FP16, FP8 = DT.float32, DT.float16, DT.float8e4

    sig_scale = -(1.0 + temp)
    sig_bias = 5.0 + theta

    nc = bacc.Bacc(
        "TRN2", target_bir_lowering=False, debug=False, enable_asserts=False,
        num_devices=N_CORES,
    )

    # ---- I/O ----
    xTf_in = nc.dram_tensor("xTf", [D_IN, N], FP16, kind="ExternalInput").ap()
    xTq_in = nc.dram_tensor("xTq", [D_IN, QR], FP16, kind="ExternalInput").ap()
    wglx_in = [
        nc.dram_tensor("wglx0", [D_IN, F0], FP16, kind="ExternalInput").ap(),
        nc.dram_tensor("wglx1", [F0, F1], FP16, kind="ExternalInput").ap(),
    ]
    bglx_in = [
        nc.dram_tensor("bglx0", [F0, 1], F32, kind="ExternalInput").ap(),
        nc.dram_tensor("bglx1", [F1, 1], F32, kind="ExternalInput").ap(),
    ]
    # negated gl weights/biases for the q side: -relu(z) = min(-W^T x - b, 0)
    wgln_in = [
        nc.dram_tensor("wgln0", [D_IN, F0], FP16, kind="ExternalInput").ap(),
        nc.dram_tensor("wgln1", [F0, F1], FP16, kind="ExternalInput").ap(),
    ]
    bgln_in = [
        nc.dram_tensor("bgln0", [F0, 1], F32, kind="ExternalInput").ap(),
        nc.dram_tensor("bgln1", [F1, 1], F32, kind="ExternalInput").ap(),
    ]
    wgna_in = [
        nc.dram_tensor("wgna0", [D_IN, F0], FP16, kind="ExternalInput").ap(),
        nc.dram_tensor("wgna1", [F0, F1], FP16, kind="ExternalInput").ap(),
    ]
    bgn0_in = nc.dram_tensor("bgn0", [F0, 1], F32, kind="ExternalInput").ap()
    zrow_in = nc.dram_tensor("zrow", [64, N], FP16, kind="ExternalInput").ap()
    y_out = nc.dram_tensor("y_out", [F1 + 1, QR], F32, kind="ExternalOutput").ap()

    with tile.TileContext(nc) as tc, ExitStack() as ctx:
        pconst = ctx.enter_context(tc.tile_pool(name="const", bufs=1))
        pouter = ctx.enter_context(tc.tile_pool(name="outer", bufs=1))
        pdram = ctx.enter_context(tc.tile_pool(name="dram", bufs=1, space="DRAM"))
        psB = ctx.enter_context(tc.tile_pool(name="psB", bufs=2, space="PSUM"))

        # ---- constants ----
        # table prime: a tiny Sqrt on a const tile makes the implicit
        # ACT_TABLE_LOAD land during startup idle instead of right before
        # the first real sqrt
        prime_in = pconst.tile([1, 2], F32, tag="prime_in")
        nc.gpsimd.memset(prime_in[:], 1.0)
        prime_out = pconst.tile([1, 2], FP16, tag="prime_out")
        nc.scalar.activation(prime_out[:], prime_in[:], AF.Sqrt, scale=1.0)
        poshalf = pconst.tile([128, 1], FP16, tag="poshalf")
        nc.gpsimd.memset(poshalf[:], 0.5)
        ones_l = pconst.tile([128, 1], FP16, tag="ones_l")   # deg weights
        nc.gpsimd.memset(ones_l[:], 1.0)
        ones_r = pconst.tile([1, 128], FP16, tag="ones_r")   # recip broadcast
        nc.gpsimd.memset(ones_r[:], 1.0)
        sgb = pconst.tile([128, 1], F32, tag="sgb")
        nc.gpsimd.memset(sgb[:], sig_bias)
        sgs = pconst.tile([128, 1], F32, tag="sgs")
        nc.gpsimd.memset(sgs[:], sig_scale)

        fins = [D_IN, F0]
        fouts = [F0, F1]
        wgl = []
        bgl = []
        wgln = []
        bgln = []
        wgna = []
        for li in range(2):
            fin, fout = fins[li], fouts[li]
            nk = fin // 128
            wk = []
            ak = []
            nwk = []
            for k in range(nk):
                t = pconst.tile([128, fout], FP16, tag=f"wgl{li}_{k}")
                nc.sync.dma_start(t[:], wglx_in[li][k * 128:(k + 1) * 128, :])
                wk.append(t)
                t = pconst.tile([128, fout], FP16, tag=f"wgna{li}_{k}")
                nc.sync.dma_start(t[:], wgna_in[li][k * 128:(k + 1) * 128, :])
                ak.append(t)
                t = pconst.tile([128, fout], FP16, tag=f"wgln{li}_{k}")
                nc.sync.dma_start(t[:], wgln_in[li][k * 128:(k + 1) * 128, :])
                nwk.append(t)
            wgl.append(wk)
            wgna.append(ak)
            wgln.append(nwk)
            bt = pconst.tile([fout, 1], F32, tag=f"bgl{li}")
            nc.sync.dma_start(bt[:], bglx_in[li][:])
            bgl.append(bt)
            bt = pconst.tile([fout, 1], F32, tag=f"bgln{li}")
            nc.sync.dma_start(bt[:], bgln_in[li][:])
            bgln.append(bt)
        bgn0 = pconst.tile([F0, 1], F32, tag="bgn0")
        nc.sync.dma_start(bgn0[:], bgn0_in[:])
        # high-half bias staged at base partition 0: the y_b accumulator
        # (features 64:127) lives on PSUM partitions 0:63, and DVE lanes
        # cannot cross partitions - the shift happens in the final DMA
        bgn0b = pconst.tile([64, 1], F32, tag="bgn0b")
        nc.sync.dma_start(bgn0b[:], bgn0_in[64:128, :])

        # layer-0 -> layer-1 bridge: instead of gathering x1 (128 rows fp16)
        # and recomputing gl1/sq/h1 per core after the gather, each core
        # computes gl1/sq1/h1 for its OWN 1024 columns locally and the
        # AllGather ships the processed tensors: rows 0:64 = gl1 (fp16),
        # row 64 = -sq1 (fp16), then h1 as fp8 bytes starting at byte
        # 65*2048, in the SAME 80-wide chunk layout h3a uses (64 h values,
        # a ones column for deg, 15 pad bytes) so both the sender and the
        # receiver DMAs move one contiguous 640-byte run per partition.
        # 217 KB on the wire instead of 256 KB, and the post-gather critical
        # chain collapses to plain DMAs.
        AGR = 106
        HBASE = 66 * 2048
        HW8 = 80
        x1Tq = pouter.tile([F0, QR], FP16, tag="x1tq")
        agin = pdram.tile([AGR, QR], FP16, tag="agin", name="agin")
        agout = pdram.tile([N_CORES * AGR, QR], FP16, tag="agout", name="agout")

        # layer-1 q-side tensors are global so the sender-side bridge work
        # can be emitted inside layer-0's qt loop (each half overlaps the
        # other half's sigmoid/adj@h) and the constant rows are filled at
        # startup, off every critical path.
        glTsq1 = pouter.tile([128, QR], FP16, tag="glTsq1")
        glshipP = pouter.tile([F1, QR], FP16, tag="glshipP")
        sqq1 = pouter.tile([1, QR], FP16, tag="sqq1")
        sqq1d = pouter.tile([1, QR], FP16, tag="sqq1d")
        gl2q1 = pouter.tile([F1, 512], FP16, tag="gl2q1")
        hstage = pouter.tile([128, 8 * HW8], FP8, tag="hstage")
        hsv = hstage.rearrange("p (c f) -> p c f", f=HW8)
        onesq1 = pouter.tile([1, QR], FP16, tag="onesq1")
        nc.gpsimd.memset(onesq1[:], 1.0)
        nc.vector.memset(glTsq1[64:128, :], 0.0)
        nc.sync.dma_start(glTsq1[65:66, :], onesq1[:])
        nc.gpsimd.memset(hstage[:], 0.0)
        nc.gpsimd.memset(hsv[:, :, 64:65], 1.0)
        # agin row 64 (the gathered glTs ones row) never changes
        nc.sync.dma_start(agin[64:65, :], onesq1[:])

        # global: the two q-half slabs and layer-0's h (its last reader is
        # the very end of layer 0, so it must never alias layer-1 tiles).
        # h is stored fp8 in two 80-wide-chunk tiles [h(0:64)|ones|pad] /
        # [h(64:128)|ones|pad]: adj@h runs as fp8 DoubleRow matmuls (K=256 =
        # two j-chunks per instruction), halving the adj@h stream count.
        # The pad columns are never read; chunk stride 80 satisfies the
        # DoubleRow step%16==0 constraint.
        slabs = [pouter.tile([128, SLABW], FP16, tag=f"slab{qt}", name=f"slab{qt}")
                 for qt in (0, 1)]
        h_a8 = pouter.tile([128, NJ * HW8], FP8, tag="h_a8")
        h_b8 = pouter.tile([128, NJ * HW8], FP8, tag="h_b8")

        prev_act = None
        last_act = [None]
        for li in range(2):
            fin, fout = fins[li], fouts[li]
            nk = fin // 128
            # layer 1 fuses both -sq rank-1 terms into the main matmul as two
            # extra contraction rows (K = 66: [gl; -sqT; 1] x [gl; 1; -sqTq]).
            # Layer 0 (fout=128) cannot, and uses a separate K=2 correction.
            fka = fout + 2 if li == 1 else fout
            # h chunk layout: layer 0 [h(0:64) | ones | h(64:128) | ones]
            # (130 wide, so adj@h splits into two M=65 stationary tiles -
            # uniform M, because alternating matmul tile sizes inside open
            # PSUM accumulation groups serializes the PE pipeline at +120ns
            # per matmul; the first ones column gives deg in row 64 of y_a,
            # the second is a dummy). Layer 1: [h | ones] (65 wide).
            hfp = fout + 2 if li == 0 else fout + 1
            jcs = list(range(NJ))
            with ExitStack() as lctx:
                pmain = lctx.enter_context(tc.tile_pool(name=f"main{li}", bufs=1))
                pacc0 = lctx.enter_context(
                    tc.tile_pool(name=f"acc0_{li}", bufs=2, space="PSUM"))
                if li == 0:
                    hta, htb = h_a8, h_b8
                else:
                    hta = pmain.tile([128, NJ * HW8], FP8, tag="h18")
                    htb = None
                h3a = hta.rearrange("p (n f) -> p n f", f=HW8)
                if li == 0:
                    # layer 1's ones column arrives with the gather
                    nc.gpsimd.memset(h3a[:, :, 64:65], 1.0)
                if htb is not None:
                    h3b = htb.rearrange("p (n f) -> p n f", f=HW8)
                    nc.gpsimd.memset(h3b[:, :, 64:65], 1.0)

                with ExitStack() as tctx:
                    # closes right after the t' loop: everything here is
                    # dead once the last t' matmul has run
                    ptp = tctx.enter_context(tc.tile_pool(name=f"tp{li}", bufs=1))
                    pst = tctx.enter_context(
                        tc.tile_pool(name=f"pst{li}", bufs=2, space="PSUM"))
                    glTs = ptp.tile([128, N], FP16, tag="glTs")
                    if li == 0:
                        glTsq = ptp.tile([128, QR], FP16, tag="glTsq")
                        sqqstage = ptp.tile([1, QR], FP16, tag="sqqstage")
                    else:
                        glTsq = glTsq1
                    # K=128 everywhere: matmuls with K in (64,128) run at
                    # 427ns vs 216ns for K=128 (measured), so the rank-2
                    # correction tensors are zero-padded to full 128 rows.
                    # Big fills run on DVE (one 4x-mode instr ~2us vs ~14us
                    # on the Q7s); tiny strided fills stay on gpsimd.
                    if li == 0:
                        # augL: row0 = -sqT, row64 = ones, rest 0
                        # augQ: row0 = ones, row64 = -sqTq, rest 0
                        augL = ptp.tile([128, N], FP16, tag="augL")
                        augQ = ptp.tile([128, QR], FP16, tag="augQ")
                        nc.vector.memset(augL[:], 0.0)
                        nc.gpsimd.memset(augL[64:65, :], 1.0)
                        nc.vector.memset(augQ[:], 0.0)
                        nc.gpsimd.memset(augQ[0:1, :], 1.0)
                    else:
                        # glTs: rows 0:64 gl, row64 = ones, row65 = -sqT,
                        # rest 0. Zero fill from a host-zeros input via DMA
                        # on the gpsimd queue - no engine time, and the DVE
                        # queue stays clear for the sender-critical
                        # evictions.
                        nc.gpsimd.dma_start(glTs[64:128, :], zrow_in[:])

                    def cstage(c, pg):
                        """evict glTs chunk c from PSUM + its -sq entries"""
                        nc.vector.tensor_scalar(
                            glTs[0:fout, c * 512:(c + 1) * 512], pg[:],
                            bgl[li][:], 0.0, ALU.add, ALU.max,
                        )
                        gl2c = pmain.tile([fout, 512], FP16, tag="gl2c")
                        nc.vector.tensor_mul(
                            gl2c[:], glTs[0:fout, c * 512:(c + 1) * 512],
                            glTs[0:fout, c * 512:(c + 1) * 512],
                        )
                        pq = psB.tile([1, 512], F32, tag="oacc")
                        nc.tensor.matmul(pq[:], poshalf[0:fout, :], gl2c[:],
                                         start=True, stop=True)
                        if li == 0:
                            nc.vector.tensor_copy(
                                augL[0:1, c * 512:(c + 1) * 512], pq[:])
                        else:
                            sqj = pmain.tile([1, 512], FP16, tag="sqj", bufs=2)
                            nc.vector.tensor_copy(sqj[:], pq[:])
                            nc.sync.dma_start(
                                glTs[65:66, c * 512:(c + 1) * 512], sqj[:])

                    # ======== setup ========
                    with ExitStack() as sctx:
                        pxt = sctx.enter_context(
                            tc.tile_pool(name=f"xt{li}", bufs=1))
                        # q side first so the first t' group unblocks early
                        if li == 0:
                            xtq = [pxt.tile([128, QR], FP16, tag=f"xtq{k}",
                                            name=f"xtq{k}") for k in range(nk)]
                            for k in range(nk):
                                nc.gpsimd.dma_start(
                                    xtq[k][:], xTq_in[k * 128:(k + 1) * 128, :])
                            for c in range(QR // 512):
                                pg = psB.tile([fout, 512], F32, tag="oacc")
                                for k in range(nk):
                                    nc.tensor.matmul(
                                        pg[:], wgln[li][k][:],
                                        xtq[k][:, c * 512:(c + 1) * 512],
                                        start=(k == 0), stop=(k == nk - 1),
                                    )
                                nc.vector.tensor_scalar(
                                    glTsq[0:fout, c * 512:(c + 1) * 512], pg[:],
                                    bgln[li][:], 0.0, ALU.add, ALU.min,
                                )
                                gl2c = pmain.tile([fout, 512], FP16, tag="gl2c")
                                nc.vector.tensor_mul(
                                    gl2c[:], glTsq[0:fout, c * 512:(c + 1) * 512],
                                    glTsq[0:fout, c * 512:(c + 1) * 512],
                                )
                                pq = psB.tile([1, 512], F32, tag="oacc")
                                nc.tensor.matmul(pq[:], poshalf[0:fout, :],
                                                 gl2c[:], start=True, stop=True)
                                nc.vector.tensor_scalar(
                                    sqqstage[0:1, c * 512:(c + 1) * 512], pq[:],
                                    DELTA, None, ALU.add)
                            nc.sync.dma_start(augQ[64:65, :], sqqstage[:])

                        if li == 0:
                            # stream xT through double-buffered 1024-col
                            # blocks; build glTs, -sq and h_nat per block
                            for b in range(8):
                                xb = [pxt.tile([128, 1024], FP16, tag=f"xtf{k}",
                                               name=f"xtf{k}", bufs=2)
                                      for k in range(nk)]
                                for k in range(nk):
                                    nc.gpsimd.dma_start(
                                        xb[k][:],
                                        xTf_in[k * 128:(k + 1) * 128,
                                               b * 1024:(b + 1) * 1024],
                                    )
                                for c4 in range(2):
                                    c = 2 * b + c4
                                    pg = psB.tile([fout, 512], F32, tag="oacc")
                                    for k in range(nk):
                                        nc.tensor.matmul(
                                            pg[:], wgl[li][k][:],
                                            xb[k][:, c4 * 512:(c4 + 1) * 512],
                                            start=(k == 0), stop=(k == nk - 1),
                                        )
                                    cstage(c, pg)
                                for g4 in range(2):
                                    j0 = 8 * b + 4 * g4
                                    ph = psB.tile([128, 4 * fout], F32, tag="oacc")
                                    for t in range(4):
                                        jl = 4 * g4 + t
                                        sl = ph[:, t * fout:(t + 1) * fout]
                                        for k in range(nk):
                                            nc.tensor.matmul(
                                                sl, xb[k][:, jl * 128:(jl + 1) * 128],
                                                wgna[li][k][:],
                                                start=(k == 0), stop=(k == nk - 1),
                                            )
                                    ph3 = ph[:].rearrange(
                                        "p (n f) -> p n f", f=fout)
                                    nc.vector.tensor_copy(
                                        h3a[:, j0:j0 + 4, 0:64], ph3[:, :, 0:64])
                                    nc.vector.tensor_copy(
                                        h3b[:, j0:j0 + 4, 0:64],
                                        ph3[:, :, 64:128])
                        else:
                            # receiver: the gather delivers processed rows;
                            # plain DMAs into glTs / h3a, no compute.
                            ao = agout[:].rearrange("(n r) c -> n r c",
                                                    n=N_CORES)
                            ao8 = agout.bitcast(FP8)[:].rearrange(
                                "a b -> (a b)")
                            h3flat = hta.rearrange("p (n w) -> p n w", w=640)
                            for r in range(N_CORES):
                                eng = nc.sync if r % 2 == 0 else nc.gpsimd
                                eng.dma_start(
                                    glTs[0:64, r * QR:(r + 1) * QR],
                                    ao[r, 0:64, :])
                                eng.dma_start(
                                    glTs[64:66, r * QR:(r + 1) * QR],
                                    ao[r, 64:66, :])
                            for r in range(N_CORES):
                                eng = nc.sync if r % 2 == 0 else nc.gpsimd
                                hb = r * AGR * 2048 + HBASE
                                eng.dma_start(
                                    h3flat[:, r, :],
                                    ao8[hb:hb + 128 * 640].rearrange(
                                        "(p w) -> p w", p=128, w=640))

                    # ======== rounds: t'+sqrt | sigmoid | adj@h ========
                    # The layer is processed in 2 rounds of 32 j-chunks.
                    # ACT runs sqrt(r0), sigmoid(r0), sqrt(r1), sigmoid(r1)
                    # back-to-back (sigmoid of one round overlaps the PE
                    # computing the next round's t'), instead of idling
                    # through each phase boundary. adj@h for qt0's round-0
                    # pairs runs during sqrt(r1); the rest chases sigmoid
                    # availability. A slice of the sqrt tiles runs on the
                    # DVE via the seed+NR path, relieving the ACT critical
                    # chain.
                    RN = 1 if li == 0 else 2
                    CR = NJ // RN
                    PR = 32 // RN
                    off_set = _offload_set(OFF_L0 if li == 0 else OFF_L1)
                    s8vs = [slabs[qt].bitcast(FP8)[:, 0:SLABW].rearrange(
                        "p (n q) -> p n q", q=512) for qt in (0, 1)]

                    def tprime_round(rnd):
                        nonlocal prev_act
                        for g0 in range(rnd * CR, (rnd + 1) * CR, 2):
                            tms = [pst.tile([128, 1024], F32, tag="tmac",
                                            name=f"tm{qt}") for qt in (0, 1)]
                            for t in range(2):
                                j = g0 + t
                                for qt in (0, 1):
                                    nc.tensor.matmul(
                                        tms[qt][:, t * 512:(t + 1) * 512],
                                        glTs[:, j * 128:(j + 1) * 128],
                                        glTsq[:, qt * 512:(qt + 1) * 512],
                                        start=True, stop=(li == 1),
                                    )
                            if li == 0:
                                for t in range(2):
                                    j = g0 + t
                                    for qt in (0, 1):
                                        nc.tensor.matmul(
                                            tms[qt][:, t * 512:(t + 1) * 512],
                                            augL[:, j * 128:(j + 1) * 128],
                                            augQ[:, qt * 512:(qt + 1) * 512],
                                            start=False, stop=True,
                                        )
                            for qt in (0, 1):
                                if (g0 // 2) * 2 + qt in off_set:
                                    # seed+NR straight from PSUM on the DVE
                                    sd = ptp.tile([128, 1024], DT.int32,
                                                  tag="nrseed",
                                                  bufs=1 if li == 0 else 2)
                                    nc.vector.tensor_scalar(
                                        sd[:], tms[qt].bitcast(DT.int32)[:, 0:1024],
                                        -0.5, K_SEED, ALU.mult, ALU.add)
                                    nc.vector._custom_dve(
                                        nrop,
                                        out=slabs[qt][:, g0 * 512:(g0 + 2) * 512],
                                        in0=sd.bitcast(F32)[:],
                                        in1=tms[qt][:, 0:1024],
                                        s0=C0_NR, s1=C1_NR)
                                    continue
                                si = nc.scalar.activation(
                                    slabs[qt][:, g0 * 512:(g0 + 2) * 512],
                                    tms[qt][:, 0:1024], AF.Sqrt,
                                )
                                if prev_act is not None:
                                    add_dep_helper(
                                        si.ins, prev_act.ins, sync=False,
                                        reason="act-table phase batching")
                                last_act[0] = si

                    def sig_round(rnd):
                        out = []
                        for qt in (0, 1):
                            slab = slabs[qt]
                            s8 = slab.bitcast(FP8)
                            lo, hi = (rnd * 32768 // RN,
                                      (rnd + 1) * 32768 // RN)
                            # fp8 out lands in the low half of the same
                            # fp16 tile; write byte trails read byte
                            si = nc.scalar.activation(
                                s8[:, lo:hi], slab[:, lo:hi], AF.Sigmoid,
                                bias=sgb[:], scale=sgs[:])
                            add_dep_helper(si.ins, last_act[0].ins, sync=False,
                                           reason="act-table phase batching")
                            out.append(si)
                        last_act[0] = out[-1]
                        return out

                    def adjh_round(qt, rnd, acc):
                        for i2 in range(rnd * PR, (rnd + 1) * PR):
                            j = 2 * i2
                            rhs2 = s8vs[qt][:, 2 * i2:2 * i2 + 2, :]
                            if li == 0:
                                nc.tensor.matmul(
                                    acc[0][:], h3a[:, j:j + 2, 0:65], rhs2,
                                    start=(i2 == 0), stop=(i2 == 31),
                                    perf_mode=mybir.MatmulPerfMode.DoubleRow,
                                )
                                nc.tensor.matmul(
                                    acc[1][:], h3b[:, j:j + 2, 0:65], rhs2,
                                    start=(i2 == 0), stop=(i2 == 31),
                                    perf_mode=mybir.MatmulPerfMode.DoubleRow,
                                )
                            else:
                                nc.tensor.matmul(
                                    acc[i2 % 2][:], h3a[:, j:j + 2, 0:65], rhs2,
                                    start=(i2 < 2), stop=(i2 >= 30),
                                    perf_mode=mybir.MatmulPerfMode.DoubleRow,
                                )

                    acc0 = (pacc0.tile([65, 512], F32, tag="acc", name="ya0"),
                            pacc0.tile([65, 512], F32, tag="acc", name="yb0"))
                    tprime_round(0)
                    sig0 = sig_round(0)
                    # HAM warm-up: self-contained M=65 matmuls into qt0's
                    # accumulator (reset by the real adj@h start right after)
                    for w in range(16):
                        mi = nc.tensor.matmul(acc0[0][:], wgna[0][0][:, 0:65],
                                              slabs[0][:, 0:512], start=True,
                                              stop=True)
                        if w == 0:
                            add_dep_helper(mi.ins, sig0[0].ins, sync=False,
                                           reason="HAM warm-up gate")
                    if RN == 2:
                        tprime_round(1)
                    adjh_round(0, 0, acc0)

                # tp/pst pools closed: their banks host qt1's accumulators
                pev = lctx.enter_context(tc.tile_pool(name=f"ev{li}", bufs=1))
                pacc1 = lctx.enter_context(
                    tc.tile_pool(name=f"acc1_{li}", bufs=2, space="PSUM"))
                acc1 = (pacc1.tile([65, 512], F32, tag="acc", name="ya1"),
                        pacc1.tile([65, 512], F32, tag="acc", name="yb1"))
                adjh_round(1, 0, acc1)
                if RN == 2:
                    sig_round(1)
                prev_act = last_act[0]

                def evict_qt(qt, acc):
                    y_a, y_b = acc
                    if li == 0:
                        degS = pev.tile([1, 512], F32, tag="degS", bufs=2)
                        nc.vector.tensor_copy(degS[:], y_a[64:65, :])
                        recipF = pev.tile([1, 512], F32, tag="recipF", bufs=2)
                        nc.vector.reciprocal_approx_fast(recipF[:], degS[:])
                        recipH = pev.tile([1, 512], FP16, tag="recipH", bufs=2)
                        nc.vector.tensor_copy(recipH[:], recipF[:])
                        rb_ps = psB.tile([128, 512], F32, tag="oacc", name="rb")
                        nc.tensor.matmul(rb_ps[:], ones_r[:], recipH[:],
                                         start=True, stop=True)
                        rbS = pev.tile([128, 512], FP16, tag="rbS", bufs=2)
                        nc.vector.tensor_copy(rbS[:], rb_ps[:])

                        x1lo = pev.tile([64, 512], FP16, tag="x1lo", bufs=2)
                        nc.vector.scalar_tensor_tensor(
                            x1lo[:], y_a[0:64, :], 1.0, rbS[0:64, :],
                            ALU.mult, ALU.mult,
                        )
                        nc.vector.tensor_scalar(
                            x1Tq[0:64, qt * 512:(qt + 1) * 512], x1lo[:],
                            bgn0[0:64, :], 0.0, ALU.add, ALU.max,
                        )
                        x1hn = pev.tile([64, 512], FP16, tag="x1hn", bufs=2)
                        nc.vector.scalar_tensor_tensor(
                            x1hn[:], y_b[0:64, :], 1.0, rbS[0:64, :],
                            ALU.mult, ALU.mult,
                        )
                        x1hi = pev.tile([64, 512], FP16, tag="x1hi", bufs=2)
                        nc.vector.tensor_scalar(
                            x1hi[:], x1hn[:], bgn0b[:], 0.0, ALU.add, ALU.max,
                        )
                        nc.sync.dma_start(
                            x1Tq[64:128, qt * 512:(qt + 1) * 512], x1hi[:])
                        # sender half of the layer bridge: gl1/sq1/h1 for
                        # this qt's 512 columns from the just-finished x1;
                        # qt0's half overlaps qt1's sigmoid/adj@h
                        pgp = psB.tile([F1, 512], F32, tag="oacc")
                        nc.tensor.matmul(pgp[:], wgl[1][0][:],
                                         x1Tq[:, qt * 512:(qt + 1) * 512],
                                         start=True, stop=True)
                        nc.vector.tensor_scalar(
                            glshipP[:, qt * 512:(qt + 1) * 512], pgp[:],
                            bgl[1][:], 0.0, ALU.add, ALU.max)
                        pgn = psB.tile([F1, 512], F32, tag="oacc")
                        nc.tensor.matmul(pgn[:], wgln[1][0][:],
                                         x1Tq[:, qt * 512:(qt + 1) * 512],
                                         start=True, stop=True)
                        nc.vector.tensor_scalar(
                            glTsq1[0:F1, qt * 512:(qt + 1) * 512], pgn[:],
                            bgln[1][:], 0.0, ALU.add, ALU.min)
                        nc.vector.tensor_mul(
                            gl2q1[:], glshipP[:, qt * 512:(qt + 1) * 512],
                            glshipP[:, qt * 512:(qt + 1) * 512])
                        pqq = psB.tile([1, 512], F32, tag="oacc")
                        nc.tensor.matmul(pqq[:], poshalf[0:F1, :], gl2q1[:],
                                         start=True, stop=True)
                        nc.vector.tensor_copy(
                            sqq1[0:1, qt * 512:(qt + 1) * 512], pqq[:])
                        nc.vector.tensor_scalar(
                            sqq1d[0:1, qt * 512:(qt + 1) * 512], pqq[:],
                            DELTA, None, ALU.add)
                        ph1 = psB.tile([128, 4 * F1], F32, tag="oacc")
                        for t in range(4):
                            jl = 4 * qt + t
                            nc.tensor.matmul(
                                ph1[:, t * F1:(t + 1) * F1],
                                x1Tq[:, jl * 128:(jl + 1) * 128],
                                wgna[1][0][:],
                                start=True, stop=True)
                        nc.vector.tensor_copy(
                            hsv[:, 4 * qt:4 * qt + 4, 0:F1],
                            ph1[:].rearrange("p (n f) -> p n f", f=F1))
                        if qt == 1:
                            nc.gpsimd.dma_start(glTsq1[64:65, :], sqq1[:])
                            nc.gpsimd.dma_start(agin[65:66, :], sqq1d[:])
                            nc.sync.dma_start(agin[0:64, :], glshipP[:])
                            agin8 = agin.bitcast(FP8)[:].rearrange(
                                "a b -> (a b)")
                            hdst = agin8[HBASE:HBASE + 128 * 640].rearrange(
                                "(p w) -> p w", p=128, w=640)
                            nc.sync.dma_start(hdst, hstage[:])
                            nc.gpsimd.collective_compute(
                                "AllGather", mybir.AluOpType.bypass,
                                ins=[agin.opt()], outs=[agout.opt()],
                                replica_groups=[list(range(N_CORES))],
                            )
                    else:
                        yhalf = pev.tile([65, 512], F32, tag="yhalf", bufs=2)
                        nc.vector.tensor_copy(yhalf[:], y_a[:])
                        yev = pev.tile([65, 512], F32, tag="yev", bufs=2)
                        nc.vector.scalar_tensor_tensor(
                            yev[:], y_b[:], 1.0, yhalf[:], ALU.mult, ALU.add,
                        )
                        nc.sync.dma_start(
                            y_out[:, qt * 512:(qt + 1) * 512], yev[:],
                        )

                if RN == 2:
                    adjh_round(0, 1, acc0)
                    evict_qt(0, acc0)
                    adjh_round(1, 1, acc1)
                    evict_qt(1, acc1)
                else:
                    evict_qt(0, acc0)
                    evict_qt(1, acc1)

    nc.compile()
    return nc


def _prep_in_maps(feat, Wgl0, bgl0, Wgnn0, bgnn0, Wgl1, bgl1, Wgnn1, bgnn1):
    s2 = np.float32(math.sqrt(2.0))
    xT = np.asarray(feat, np.float32).T

    def f32(a):
        return np.asarray(a, np.float32)

    xT16 = np.ascontiguousarray(xT.astype(np.float16))
    wglx0 = np.ascontiguousarray((f32(Wgl0) * s2).astype(np.float16))
    bglx0 = np.ascontiguousarray((f32(bgl0) * s2).reshape(-1, 1))
    wglx1 = np.ascontiguousarray((f32(Wgl1) * s2).astype(np.float16))
    bglx1 = np.ascontiguousarray((f32(bgl1) * s2).reshape(-1, 1))
    wgln0 = np.ascontiguousarray(-wglx0)
    bgln0 = np.ascontiguousarray(-bglx0)
    wgln1 = np.ascontiguousarray(-wglx1)
    bgln1 = np.ascontiguousarray(-bglx1)
    wgna0 = np.ascontiguousarray(f32(Wgnn0).astype(np.float16))
    wgna1 = np.ascontiguousarray(f32(Wgnn1).astype(np.float16))
    bgn0 = np.ascontiguousarray(f32(bgnn0).reshape(-1, 1))

    zrow = np.zeros((64, N), np.float16)
    in_maps = []
    for r in range(N_CORES):
        in_maps.append({
            "xTf": xT16,
            "xTq": np.ascontiguousarray(xT16[:, r * QR:(r + 1) * QR]),
            "wglx0": wglx0, "bglx0": bglx0, "wgna0": wgna0,
            "wglx1": wglx1, "bglx1": bglx1, "wgna1": wgna1,
            "wgln0": wgln0, "bgln0": bgln0,
            "wgln1": wgln1, "bgln1": bgln1,
            "bgn0": bgn0, "zrow": zrow,
        })
    return in_maps


def _postprocess(results, bgnn1):
    y = np.concatenate(
        [np.asarray(results[r]["y_out"]).T for r in range(N_CORES)], axis=0
    )  # [8192, 65]
    x2 = y[:, :F1] / y[:, F1:F1 + 1] + np.asarray(bgnn1, np.float32).reshape(1, -1)
    m = x2.max(axis=-1, keepdims=True)
    e = np.exp(x2 - m)
    return (e / e.sum(axis=-1, keepdims=True)).astype(np.float32)


def kernel(**inputs):
    from concourse.bass_utils import run_bass_kernel_spmd

    feat = np.asarray(inputs["feat_matrix"], np.float32)
    temp = float(np.asarray(inputs["temp"]))
    theta = float(np.asarray(inputs["theta"]))
    key = (round(temp, 9), round(theta, 9))
    if key not in _CACHE:
        _CACHE[key] = _build(temp, theta)
    nc = _CACHE[key]

    in_maps = _prep_in_maps(
        feat, inputs["Wgl0"], inputs["bgl0"], inputs["Wgnn0"], inputs["bgnn0"],
        inputs["Wgl1"], inputs["bgl1"], inputs["Wgnn1"], inputs["bgnn1"],
    )
    res = run_bass_kernel_spmd(nc, in_maps, list(range(N_CORES)))
    return _postprocess(res.results, inputs["bgnn1"])

